# revision 1
# baseline (speedup 1.0000x reference)
"""Trainium2 Bass kernel for nn_Choquet_Integral.

Reformulation: the Choquet integral (sort + successive diffs + FM lattice
gather + einsum) equals a Mobius-transform contraction over subset minima:

    y[b, h] = sum_{T subset of {0..7}, T nonempty} mHat[T, h] * min_{i in T} x_b[i]

where mHat is the Mobius transform of the fuzzy measure FM (host-computed,
255 x 8). Subset minima are produced with min(a,b) = (a + b - |a - b|)/2 in a
3-level balanced cascade, so everything becomes constant-matrix matmuls (PE)
interleaved with elementwise |.| (ACT/DVE). No sort, no gather.

Stages per sample n (b = the 512 d-columns, free dim):
  Z[0:8]   = x rows (DMA)
  S1 (PE): D2 = A1^T Z[0:8]            -> |.| -> Z[8:12]
  S2 (PE): D4 = A2^T Z[0:12]           -> |.| -> Z[12:30]
  S3 (PE): D8 = A3^T Z[0:30] (225 rows)-> |.| -> W8a/W8b
  y0 (PE): y  = C030^T Z[0:30]
  S4 (PE): y += C8a^T W8a + C8b^T W8b  (PSUM accumulation)
Then per 16 samples: LayerNorm over (H, D) + PReLU, batched on [128, 512]
tiles (per-sample stats via a block-diagonal ones matmul), DMA out.

Sharding: data-parallel over N across the 8 NeuronCores (256 samples each).
"""

import sys

for _p in ("/opt/trn_rl_repo", "/root/.axon_site/_ro/trn_rl_repo"):
    if _p not in sys.path:
        sys.path.append(_p)

import numpy as np

import concourse.bass as bass
import concourse.bacc as bacc
import concourse.tile as tile
from concourse import mybir
from concourse.tile_rust import add_dep_helper
from concourse.bass_utils import run_bass_kernel_spmd

N, S, D, H = 2048, 8, 512, 8
NCORES = 8
NPC = N // NCORES  # samples per core
LN_EPS = 1e-5
F32 = mybir.dt.float32
F32R = mybir.dt.float32r

NZ = 255  # Z feature rows: 8 x | 4 |d2| | 18 |d4| | 225 |d8|
# 2-sample-paired on-chip Z tile: x(A,B)@0:16, R2(A,B)@32:40, R4(A,B)@64:100
ZROWS = 100


# --------------------------------------------------------------------------
# Host-side constant matrices
# --------------------------------------------------------------------------
def _build_structure():
    """FM-independent pieces: A1 [8,4], A2 [12,18], A3 [30,225], and the
    linear forms of every subset minimum over the 255-dim Z vector."""

    def v_x(i):
        v = np.zeros(NZ)
        v[i] = 1.0
        return v

    def e(row):
        v = np.zeros(NZ)
        v[row] = 1.0
        return v

    # relu convention: min(a, b) = a - relu(a - b); row e(.) holds relu(diff)
    m2 = [v_x(2 * p) - e(8 + p) for p in range(4)]

    def P(p, a):  # pair p value for local mask a in {1,2,3}
        return (v_x(2 * p), v_x(2 * p + 1), m2[p])[a - 1]

    m4 = {0: {}, 1: {}}
    d4rows = {0: {}, 1: {}}
    for side in range(2):
        p0, p1 = (0, 1) if side == 0 else (2, 3)
        for t in range(1, 16):
            a, b = t & 3, t >> 2
            if b == 0:
                m4[side][t] = P(p0, a)
            elif a == 0:
                m4[side][t] = P(p1, b)
            else:
                d4rows[side][(a, b)] = P(p0, a) - P(p1, b)
                m4[side][t] = P(p0, a) - e(12 + 9 * side + 3 * (a - 1) + (b - 1))

    d8rows = {}
    minT = {}
    for T in range(1, 256):
        t, u = T & 15, T >> 4
        if u == 0:
            minT[T] = m4[0][t]
        elif t == 0:
            minT[T] = m4[1][u]
        else:
            d8rows[(t, u)] = m4[0][t] - m4[1][u]
            minT[T] = m4[0][t] - e(30 + 15 * (t - 1) + (u - 1))

    A1 = np.zeros((8, 4))
    for p in range(4):
        A1[2 * p, p] = 1.0
        A1[2 * p + 1, p] = -1.0

    A2 = np.zeros((12, 18))
    for side in range(2):
        for a in range(1, 4):
            for b in range(1, 4):
                A2[:, 9 * side + 3 * (a - 1) + (b - 1)] = d4rows[side][(a, b)][:12]

    A3 = np.zeros((30, 225))
    for t in range(1, 16):
        for u in range(1, 16):
            A3[:, 15 * (t - 1) + (u - 1)] = d8rows[(t, u)][:30]

    return A1, A2, A3, minT


_A1, _A2, _A3, _MINT = _build_structure()


def _mobius(FM):
    """mHat[T, h], T in [0, 255]; mu(mask) = FM[mask-1], mu(0) = 0."""
    mh = np.zeros((256, H), np.float64)
    mh[1:] = FM.astype(np.float64)
    for b in range(8):
        bit = 1 << b
        idx = np.arange(256)
        hi = idx[(idx & bit) != 0]
        mh[hi] -= mh[hi ^ bit]
    return mh


def _host_matrices(FM):
    mh = _mobius(FM)
    C = np.zeros((NZ, H))
    for T in range(1, 256):
        C += np.outer(_MINT[T], mh[T])
    f = np.float32

    # Samples are processed in PAIRS sharing one Z tile [100, D]:
    #   sample A rows: x@0:8,  R2@32:36, R4@64:82
    #   sample B rows: x@8:16, R2@36:40, R4@82:100
    # (engine ops may change partition base between in/out but only at
    # 32-aligned bases; f32r matmuls must write PSUM at base 0)
    def zscatter2(M, par):
        out = np.zeros((ZROWS, M.shape[1]))
        o = 8 * par
        out[o : o + 8] = M[0:8]
        out[32 + 4 * par : 36 + 4 * par] = M[8:12]
        out[64 + 18 * par : 82 + 18 * par] = M[12:30]
        return out

    # S1 pair matrix: [16, 8] block-diag of A1
    A1q = np.zeros((16, 8))
    A1q[0:8, 0:4] = _A1
    A1q[8:16, 4:8] = _A1

    # S2 pair matrix: [40, 36]; K rows = x(A,B)@0:16 + R2(A,B)@32:40 -> slice
    A2q = np.zeros((40, 36))
    A2q[0:8, 0:18] = _A2[0:8]
    A2q[32:36, 0:18] = _A2[8:12]
    A2q[8:16, 18:36] = _A2[0:8]
    A2q[36:40, 18:36] = _A2[8:12]

    # f32r matmuls must write PSUM at partition base 0, so per-sample y
    # outputs (8 rows) are emitted as full-bank M=128 matmuls: 16 slot
    # variants with the C columns placed at columns 8k..8k+8 of zeros.
    def slotted(Cpart, rows=None):
        Kr = Cpart.shape[0]
        out = np.zeros((16, Kr, 128), f)
        for k in range(16):
            out[k, :, 8 * k : 8 * k + 8] = Cpart
        return out

    mats = {
        "a1": A1q.astype(f),
        "a2": A2q.astype(f),
        "g16": _g16(),
    }
    for par in range(2):
        A3p = zscatter2(_A3, par)
        mats[f"a3a{par}"] = A3p[:, 0:128].astype(f)
        mats[f"a3b{par}"] = A3p[:, 128:225].astype(f)
    c030v = np.zeros((16, ZROWS, 128), f)
    for k in range(16):
        c030v[k][:, 8 * k : 8 * k + 8] = zscatter2(C[0:30, :], k % 2)
    mats["c030v"] = c030v
    mats["c8av"] = slotted(C[30:158, :])
    mats["c8bv"] = slotted(C[158:255, :])
    return mats


def _g16():
    """Block-diagonal ones [128, 128]: per-sample (8-row group) sum replicator."""
    g = np.zeros((128, 128), np.float32)
    for k in range(16):
        g[8 * k : 8 * k + 8, 8 * k : 8 * k + 8] = 1.0
    return g


# cpack column layout: every constant packed into one [128, CP] f32 tensor so
# the whole preamble is a single DMA (keeps the drain sync-wait count low).
_CPCOLS = {
    "a1": (0, 8, 16),
    "a2": (8, 36, 40),
    "a3a0": (44, 128, ZROWS),
    "a3a1": (172, 128, ZROWS),
    "a3b0": (300, 97, ZROWS),
    "a3b1": (397, 97, ZROWS),
    "c030v": (494, 128, ZROWS),  # 16 slots of 128
    "c8av": (2542, 128, 128),
    "c8bv": (4590, 128, 97),
    "g16": (6638, 128, 128),
    "lnw": (6766, D, 128),
    "lnb": (7278, D, 128),
    "pre": (7790, 1, 128),
    "eps": (7791, 1, 128),
}
CP = 7792


def _pack_consts(mats, lnw, lnb, pre_w):
    cp = np.zeros((128, CP), np.float32)

    def put(name, arr, slot=None):
        c0, w, rows = _CPCOLS[name]
        if slot is not None:
            c0 += 128 * slot
        cp[: arr.shape[0], c0 : c0 + arr.shape[1]] = arr

    put("a1", mats["a1"])
    put("a2", mats["a2"])
    for par in range(2):
        put(f"a3a{par}", mats[f"a3a{par}"])
        put(f"a3b{par}", mats[f"a3b{par}"])
    for k in range(16):
        put("c030v", mats["c030v"][k], slot=k)
        put("c8av", mats["c8av"][k], slot=k)
        put("c8bv", mats["c8bv"][k], slot=k)
    put("g16", _g16())
    put("lnw", lnw)
    put("lnb", lnb)
    cp[:, _CPCOLS["pre"][0]] = pre_w
    cp[:, _CPCOLS["eps"][0]] = LN_EPS
    return cp


# --------------------------------------------------------------------------
# Bass module
# --------------------------------------------------------------------------
def build_module(npc=NPC, mm_dtype=F32R):
    nc = bacc.Bacc("TRN2", target_bir_lowering=False, debug=False)

    x_in = nc.dram_tensor("x", [npc, S, D], mm_dtype, kind="ExternalInput").ap()
    y_out = nc.dram_tensor("y", [npc, H, D], F32, kind="ExternalOutput").ap()

    cpack = nc.dram_tensor("cpack", [128, CP], mm_dtype, kind="ExternalInput").ap()

    AluOp = mybir.AluOpType
    Act = mybir.ActivationFunctionType

    def mm(out, lhsT, rhs, **kw):
        nc.tensor.matmul(out, lhsT, rhs, **kw)

    # ---- persistent SBUF constants + Z buffers ----
    cpk = nc.alloc_sbuf_tensor("cpk", [128, CP], mm_dtype).ap()

    def cslice(name, slot=None, bitcast=None):
        c0, w, rows = _CPCOLS[name]
        if slot is not None:
            c0 += 128 * slot
        ap = cpk[0:rows, c0 : c0 + w]
        return ap.bitcast(bitcast) if bitcast is not None else ap

    ct = {
        "a1": cslice("a1"),
        "a2": cslice("a2"),
        "a3a0": cslice("a3a0"),
        "a3a1": cslice("a3a1"),
        "a3b0": cslice("a3b0"),
        "a3b1": cslice("a3b1"),
        "g16": cslice("g16", bitcast=F32),
        "lnw": cslice("lnw", bitcast=F32),
        "lnb": cslice("lnb", bitcast=F32),
    }
    for k in range(16):
        ct[f"c030v{k}"] = cslice("c030v", slot=k)
        ct[f"c8av{k}"] = cslice("c8av", slot=k)
        ct[f"c8bv{k}"] = cslice("c8bv", slot=k)
    pre = cslice("pre", bitcast=F32)
    eps = cslice("eps", bitcast=F32)
    NZB = 8
    z_bufs = [
        nc.alloc_sbuf_tensor(f"zbuf{i}", [ZROWS, D], mm_dtype).ap() for i in range(NZB)
    ]
    # pair-wide S3 PSUM tensors, double-buffered: two banks each (columns
    # 0:512 = sample A, 512:1024 = sample B) so one relu op drains both
    # samples.  NOTE: with d24+y16 this uses all 8 PSUM banks.
    p8a_pairs = [nc.alloc_psum_tensor(f"p8apair{i}", [128, 2 * D], F32).ap() for i in range(1)]
    p8b_pairs = [nc.alloc_psum_tensor(f"p8bpair{i}", [97, 2 * D], F32).ap() for i in range(1)]

    # Preamble TileContext: one const DMA + Z-buffer zeroing; its exit barrier
    # fully separates these deps from the main loop.
    with tile.TileContext(nc) as tc0:
        nc.sync.dma_start(out=cpk, in_=cpack)
        for zb in z_bufs:
            nc.gpsimd.memset(zb[:, :].bitcast(F32), 0.0)

    with tile.TileContext(nc) as tc:
        with (
            tc.tile_pool(name="wpool", bufs=4) as wpool,
            tc.tile_pool(name="lnpool", bufs=3) as lnpool,
            tc.tile_pool(name="ps_small", bufs=2, space="PSUM") as ps_small,
            tc.tile_pool(name="ps_big", bufs=1, space="PSUM") as ps_big,
            tc.tile_pool(name="ps_y", bufs=2, space="PSUM") as ps_y,
        ):
            def emit_ln_stats(blkp, y16p):
                    # ---- LayerNorm over (H, D) per sample + PReLU, batched ----
                    r = lnpool.tile([128, 2], F32, tag="r")
                    sq = lnpool.tile([128, D], F32, tag="sq")
                    nc.vector.tensor_reduce(
                        out=r[:, 0:1], in_=y16p[:], axis=mybir.AxisListType.X, op=AluOp.add
                    )
                    nc.scalar.activation(
                        out=sq[:], in_=y16p[:], func=Act.Square, accum_out=r[:, 1:2]
                    )
                    sp = ps_small.tile([128, 2], F32, tag="d24")
                    nc.tensor.matmul(sp[:], ct["g16"], r[:])

                    mean = lnpool.tile([128, 1], F32, tag="mean")
                    var = lnpool.tile([128, 1], F32, tag="var")
                    rstd = lnpool.tile([128, 1], F32, tag="rstd")
                    nc.vector.tensor_scalar(
                        out=mean[:], in0=sp[:, 0:1], scalar1=1.0 / (H * D), scalar2=None,
                        op0=AluOp.mult,
                    )
                    # var = sp[:,1]/HD - mean^2  (guard: compute mean^2 first)
                    m2t = lnpool.tile([128, 1], F32, tag="m2t")
                    nc.vector.tensor_tensor(
                        out=m2t[:], in0=mean[:], in1=mean[:], op=AluOp.mult
                    )
                    nc.vector.scalar_tensor_tensor(
                        out=var[:], in0=sp[:, 1:2], scalar=1.0 / (H * D),
                        in1=m2t[:], op0=AluOp.mult, op1=AluOp.subtract,
                    )
                    # rstd = 1/sqrt(var + eps)
                    nc.scalar.activation(out=rstd[:], in_=var[:], func=Act.Sqrt, bias=eps)
                    nc.vector.reciprocal(out=rstd[:], in_=rstd[:])

                    return mean, rstd

            def emit_ln_apply(blkp, y16p, mean, rstd):
                    t1 = lnpool.tile([128, D], F32, tag="t1")
                    nc.vector.tensor_scalar(
                        out=t1[:], in0=y16p[:], scalar1=mean[:], scalar2=rstd[:],
                        op0=AluOp.subtract, op1=AluOp.mult,
                    )
                    t2 = lnpool.tile([128, D], F32, tag="t2")
                    nc.gpsimd.tensor_tensor(out=t2[:], in0=t1[:], in1=ct["lnw"], op=AluOp.mult)
                    nc.gpsimd.tensor_tensor(out=t2[:], in0=t2[:], in1=ct["lnb"], op=AluOp.add)
                    # prelu: out = max(t2, 0) + pre * min(t2, 0)
                    u = lnpool.tile([128, D], F32, tag="u")
                    nc.vector.tensor_scalar(
                        out=u[:], in0=t2[:], scalar1=0.0, scalar2=pre,
                        op0=AluOp.min, op1=AluOp.mult,
                    )
                    o16 = lnpool.tile([128, D], F32, tag="o16")
                    nc.vector.scalar_tensor_tensor(
                        out=o16[:], in0=t2[:], scalar=0.0, in1=u[:],
                        op0=AluOp.max, op1=AluOp.add,
                    )
                    nc.gpsimd.dma_start(
                        out=y_out[blkp * 16 : blkp * 16 + 16].rearrange("n h d -> (n h) d"),
                        in_=o16[:],
                    )



            pending_ln = None
            for blk in range(npc // 16):
                # one full PSUM bank accumulates y for 16 samples (8 rows each)
                y16 = ps_y.tile([128, D], F32, tag="y16")
                def front(j):
                    # x DMA + paired S1/S2 diff chain; emitted one pair ahead
                    # so the serial S1->relu->S2->relu latency hides under the
                    # previous pair's S3/y matmul work
                    n0 = blk * 16 + 2 * j
                    z = z_bufs[(blk * 8 + j) % NZB]
                    nc.gpsimd.dma_start(
                        out=z[0:16, :],
                        in_=x_in[n0 : n0 + 2].rearrange("n s d -> (n s) d"),
                    )
                    d24 = ps_small.tile([36, D], F32, tag="d24")
                    mm(d24[0:8, :], ct["a1"], z[0:16, :])
                    nc.vector.tensor_scalar(
                        out=z[32:40, :], in0=d24[0:8, :], scalar1=0.0, scalar2=None,
                        op0=AluOp.max,
                    )
                    mm(d24[0:36, :], ct["a2"], z[0:40, :])
                    nc.scalar.activation(out=z[64:100, :], in_=d24[0:36, :], func=Act.Relu)
                    return z

                zs = {}
                for j in range(8):  # pairs of samples
                    if j == 0:
                        zs[0] = front(0)
                    if j + 1 < 8:
                        zs[j + 1] = front(j + 1)
                    z = zs[j]
                    if j == 2 and pending_ln is not None:
                        # deferred LayerNorm of the previous block: by now its
                        # upstream (stats matmul etc.) inputs are long done, so
                        # the chain never head-of-line blocks DVE/ACT
                        ln_mid = emit_ln_stats(*pending_ln)
                    if j == 5 and pending_ln is not None:
                        emit_ln_apply(*pending_ln, *ln_mid)
                        pending_ln = None

                    # software-pipeline within the pair: all S3 matmuls and
                    # the (fused, pair-wide) relus first, then relu-independent
                    # c030v matmuls, then the relu-consuming y accumulations.
                    p8a_pair = p8a_pairs[j % len(p8a_pairs)]
                    p8b_pair = p8b_pairs[j % len(p8b_pairs)]
                    w8a = wpool.tile([128, 2 * D], mm_dtype, tag="w8a")
                    w8b = wpool.tile([97, 2 * D], mm_dtype, tag="w8b")
                    for par in range(2):
                        cs = slice(par * D, par * D + D)
                        mm(p8a_pair[:, cs], ct[f"a3a{par}"], z[:])
                        mm(p8b_pair[:, cs], ct[f"a3b{par}"], z[:])
                    nc.scalar.activation(out=w8a[:], in_=p8a_pair[:, :], func=Act.Relu)
                    nc.vector.tensor_scalar(
                        out=w8b[:], in0=p8b_pair[:, :], scalar1=0.0, scalar2=None,
                        op0=AluOp.max,
                    )
                    for par in range(2):
                        k = 2 * j + par
                        mm(y16[:], ct[f"c030v{k}"], z[:], start=(k == 0), stop=False)
                    for par in range(2):
                        k = 2 * j + par
                        cs = slice(par * D, par * D + D)
                        mm(y16[:], ct[f"c8av{k}"], w8a[:, cs], start=False, stop=False)
                        mm(y16[:], ct[f"c8bv{k}"], w8b[:, cs], start=False, stop=(k == 15))

                pending_ln = (blk, y16)
            emit_ln_apply(*pending_ln, *emit_ln_stats(*pending_ln))

    nc.compile()
    return nc


# --------------------------------------------------------------------------
# Entry point
# --------------------------------------------------------------------------
_CACHED = {}


def _get_module():
    if "nc" not in _CACHED:
        _CACHED["nc"] = build_module()
    return _CACHED["nc"]


def host_feeds(FM, ln_weight, ln_bias, prelu_w):
    mats = _host_matrices(np.asarray(FM, np.float64))
    lnw = np.tile(np.asarray(ln_weight, np.float32).reshape(1, H, D), (16, 1, 1)).reshape(128, D)
    lnb = np.tile(np.asarray(ln_bias, np.float32).reshape(1, H, D), (16, 1, 1)).reshape(128, D)
    return _pack_consts(mats, lnw, lnb, float(np.asarray(prelu_w).reshape(-1)[0]))


def kernel(x, FM, ln_weight, ln_bias, prelu_w):
    x = np.ascontiguousarray(np.asarray(x, np.float32))
    cpack = np.ascontiguousarray(host_feeds(FM, ln_weight, ln_bias, prelu_w))

    nc = _get_module()
    in_maps = []
    for c in range(NCORES):
        in_maps.append(
            {"x": np.ascontiguousarray(x[c * NPC : (c + 1) * NPC]), "cpack": cpack}
        )

    res = run_bass_kernel_spmd(nc, in_maps, core_ids=list(range(NCORES)))
    out = np.concatenate([r["y"] for r in res.results], axis=0)
    return out.astype(np.float32)


if __name__ == "__main__":
    # smoke-test build
    nc = build_module(npc=16)
    print("module built ok")



# revision 16
# speedup vs baseline: 1.2771x; 1.2771x over previous
"""Trainium2 Bass kernel for nn_Choquet_Integral.

Mobius-transform reformulation (see kernel_baseline.py for derivation):

    y[b, h] = sum_{T subset of {0..7}, T nonempty} mHat[T, h] * min_{i in T} x_b[i]

Subset minima come from a relu-difference cascade (S1: 4 pair diffs, S2: 18
quad diffs, S3: 225 cross diffs), all constant-matrix matmuls.  This version
minimizes PE instruction count (the cost model charges a matmul by its output
free size only):

  - quad z tiles [120, 512] hold 4 samples' x/R2/R4 rows, so S1, S2 and the
    z30 part of the y-contraction (c030) are ONE matmul per 4 samples each.
  - S3 is 4 matmuls per pair (450 feature rows, M<=128), into pair-wide
    PSUM [128,1024]+[97,1024]; relu drains write bf16 feature tiles.
  - y-contraction of the 450 cross features: 4 dense matmuls per pair
    (K=128/97), slotted into a block-wide y16 PSUM bank (16 samples x 8 H).
  - matmul dtype is f32r (bf16 intermediates lose too much: cascade
    rounding is amplified ~8x by the Mobius coefficients).

Per pair: 8.5 matmuls x 512 cols vs baseline's 12.  LayerNorm/PReLU per
16-sample block, batched [128, 512]; ln_weight/ln_bias application is skipped
when they are ones/zeros (checked at runtime on host).

Sharding: data-parallel over N across 8 NeuronCores (256 samples each).
"""

import sys

for _p in ("/opt/trn_rl_repo", "/root/.axon_site/_ro/trn_rl_repo"):
    if _p not in sys.path:
        sys.path.append(_p)

import ml_dtypes
import numpy as np

import concourse.bass as bass
import concourse.bacc as bacc
import concourse.tile as tile
from concourse import mybir
from concourse.bass_utils import run_bass_kernel_spmd

N, S, D, H = 2048, 8, 512, 8
NCORES = 8
NPC = N // NCORES  # samples per core
LN_EPS = 1e-5
F32 = mybir.dt.float32
F32R = mybir.dt.float32r

ZQ = 120  # quad z tile rows: 4 samples x (x 8 | R2 4 | R4 18)


def zqrow(s, i):
    """Row of sample s's z30-basis element i inside the quad z tile.

    Engine partition windows must be naturally aligned: size>64 -> base 0,
    size<=32 -> base in {0,32,64,96}.  Layout:
      R4 rows 0:72   (S2 psum [0:72] -> zq [0:72], base 0)
      x  rows 72:96 (samples 0,1,2) and 112:120 (sample 3)  [DMA, 2 slabs]
      R2 rows 96:112 (S1 psum [0:16] -> zq [96:112], 16@96)
    """
    if i < 8:
        return (72 + 8 * s + i) if s < 3 else (112 + i)
    if i < 12:
        return 96 + 4 * s + (i - 8)
    return 18 * s + (i - 12)


# --------------------------------------------------------------------------
# Host-side constant matrices
# --------------------------------------------------------------------------
def _build_structure():
    """FM-independent pieces: A1 [8,4], A2 [12,18], A3 [30,225], and the
    linear forms of every subset minimum over the 255-dim feature vector."""
    NZ = 255

    def v_x(i):
        v = np.zeros(NZ)
        v[i] = 1.0
        return v

    def e(row):
        v = np.zeros(NZ)
        v[row] = 1.0
        return v

    # relu convention: min(a, b) = a - relu(a - b); row e(.) holds relu(diff)
    m2 = [v_x(2 * p) - e(8 + p) for p in range(4)]

    def P(p, a):  # pair p value for local mask a in {1,2,3}
        return (v_x(2 * p), v_x(2 * p + 1), m2[p])[a - 1]

    m4 = {0: {}, 1: {}}
    d4rows = {0: {}, 1: {}}
    for side in range(2):
        p0, p1 = (0, 1) if side == 0 else (2, 3)
        for t in range(1, 16):
            a, b = t & 3, t >> 2
            if b == 0:
                m4[side][t] = P(p0, a)
            elif a == 0:
                m4[side][t] = P(p1, b)
            else:
                d4rows[side][(a, b)] = P(p0, a) - P(p1, b)
                m4[side][t] = P(p0, a) - e(12 + 9 * side + 3 * (a - 1) + (b - 1))

    d8rows = {}
    minT = {}
    for T in range(1, 256):
        t, u = T & 15, T >> 4
        if u == 0:
            minT[T] = m4[0][t]
        elif t == 0:
            minT[T] = m4[1][u]
        else:
            d8rows[(t, u)] = m4[0][t] - m4[1][u]
            minT[T] = m4[0][t] - e(30 + 15 * (t - 1) + (u - 1))

    A1 = np.zeros((8, 4))
    for p in range(4):
        A1[2 * p, p] = 1.0
        A1[2 * p + 1, p] = -1.0

    A2 = np.zeros((12, 18))
    for side in range(2):
        for a in range(1, 4):
            for b in range(1, 4):
                A2[:, 9 * side + 3 * (a - 1) + (b - 1)] = d4rows[side][(a, b)][:12]

    A3 = np.zeros((30, 225))
    for t in range(1, 16):
        for u in range(1, 16):
            A3[:, 15 * (t - 1) + (u - 1)] = d8rows[(t, u)][:30]

    return A1, A2, A3, minT


_A1, _A2, _A3, _MINT = _build_structure()


def _mobius(FM):
    """mHat[T, h], T in [0, 255]; mu(mask) = FM[mask-1], mu(0) = 0."""
    mh = np.zeros((256, H), np.float64)
    mh[1:] = FM.astype(np.float64)
    for b in range(8):
        bit = 1 << b
        idx = np.arange(256)
        hi = idx[(idx & bit) != 0]
        mh[hi] -= mh[hi ^ bit]
    return mh


# S3 chunk layout: (name, sample-in-pair, col offset into _A3/C8, rows)
_CHUNKS = [("a0", 0, 0, 128), ("b0", 1, 0, 128), ("a1", 0, 128, 97), ("b1", 1, 128, 97)]


def _host_matrices(FM):
    mh = _mobius(FM)
    C = np.zeros((255, H))
    for T in range(1, 256):
        C += np.outer(_MINT[T], mh[T])
    # C rows: 0:8 x, 8:12 R2, 12:30 R4, 30:255 cross (d8) features

    mats = {}
    # S1 quad: [120, 16] full-window lhsT (zero rows cost nothing)
    a1q = np.zeros((ZQ, 16))
    for s in range(4):
        for p in range(4):
            a1q[zqrow(s, 2 * p), 4 * s + p] = 1.0
            a1q[zqrow(s, 2 * p + 1), 4 * s + p] = -1.0
    mats["a1q"] = a1q

    # S2 quad: [120, 72]
    a2q = np.zeros((ZQ, 72))
    for s in range(4):
        for j in range(18):
            for i in range(12):
                a2q[zqrow(s, i), 18 * s + j] = _A2[i, j]
    mats["a2q"] = a2q

    # S3 chunks, per pair-in-quad p
    for p in range(2):
        for name, sp, off, rows in _CHUNKS:
            m = np.zeros((ZQ, rows))
            for i in range(30):
                m[zqrow(2 * p + sp, i), :] = _A3[i, off : off + rows]
            mats[f"a3_{p}_{name}"] = m

    # c030 quad: [120, 128] per quad-position Q in block
    for Q in range(4):
        m = np.zeros((ZQ, 128))
        for s in range(4):
            j = 4 * Q + s
            for i in range(30):
                m[zqrow(s, i), 8 * j : 8 * j + 8] = C[i, :]
        mats[f"c030q{Q}"] = m

    # y cross-feature weights, slotted per block sample j
    for j in range(16):
        ma = np.zeros((128, 128))
        ma[:, 8 * j : 8 * j + 8] = C[30:158, :]
        mats[f"c8a{j}"] = ma
        mb = np.zeros((97, 128))
        mb[:, 8 * j : 8 * j + 8] = C[158:255, :]
        mats[f"c8b{j}"] = mb
    return mats


def _pack_consts16(mats):
    """Pack all bf16 lhsT constants into one [128, CP16] tensor; returns
    (packed array, {name: (col0, ncols, nrows)})."""
    order = (
        ["a1q", "a2q"]
        + [f"a3_{p}_{c[0]}" for p in range(2) for c in _CHUNKS]
        + [f"c030q{Q}" for Q in range(4)]
        + [f"c8a{j}" for j in range(16)]
        + [f"c8b{j}" for j in range(16)]
    )
    cols = {}
    c0 = 0
    for name in order:
        m = mats[name]
        cols[name] = (c0, m.shape[1], m.shape[0])
        c0 += m.shape[1]
    cp = np.zeros((128, c0), np.float32)
    for name in order:
        col, w, rows = cols[name]
        cp[:rows, col : col + w] = mats[name].astype(np.float32)
    return cp, cols


def _g16():
    """Block-diagonal ones [128, 128]: per-sample (8-row group) sum replicator."""
    g = np.zeros((128, 128), np.float32)
    for k in range(16):
        g[8 * k : 8 * k + 8, 8 * k : 8 * k + 8] = 1.0
    return g


# cpack32 layout (f32): g16 [128,128] | pre [128,1] | eps [128,1] | lnw | lnb
_CP32 = {"g16": (0, 128), "pre": (128, 1), "eps": (129, 1), "lnw": (130, D), "lnb": (130 + D, D)}
CP32 = 130 + 2 * D


def _pack_consts32(lnw, lnb, pre_w):
    cp = np.zeros((128, CP32), np.float32)
    cp[:, 0:128] = _g16()
    cp[:, 128] = pre_w
    cp[:, 129] = LN_EPS
    cp[:, 130 : 130 + D] = lnw
    cp[:, 130 + D : 130 + 2 * D] = lnb
    return cp


# --------------------------------------------------------------------------
# Bass module
# --------------------------------------------------------------------------
def build_module(npc=NPC, apply_affine=False, cols16=None, cp16_width=None):
    nc = bacc.Bacc("TRN2", target_bir_lowering=False, debug=False)

    x_in = nc.dram_tensor("x16", [npc, S, D], F32R, kind="ExternalInput").ap()
    y_out = nc.dram_tensor("y", [npc, H, D], F32, kind="ExternalOutput").ap()
    cpack16 = nc.dram_tensor("cpack16", [128, cp16_width], F32R, kind="ExternalInput").ap()
    cpack32 = nc.dram_tensor("cpack32", [128, CP32], F32, kind="ExternalInput").ap()

    AluOp = mybir.AluOpType
    Act = mybir.ActivationFunctionType

    def mm(out, lhsT, rhs, **kw):
        nc.tensor.matmul(out, lhsT, rhs, **kw)

    cpk16 = nc.alloc_sbuf_tensor("cpk16", [128, cp16_width], F32R).ap()
    cpk32 = nc.alloc_sbuf_tensor("cpk32", [128, CP32], F32).ap()

    def ct(name):
        col, w, rows = cols16[name]
        return cpk16[0:rows, col : col + w]

    def c32(name):
        col, w = _CP32[name]
        return cpk32[0:128, col : col + w]

    g16 = c32("g16")
    pre = c32("pre")
    eps = c32("eps")
    lnw = c32("lnw")
    lnb = c32("lnb")

    NQUADS = npc // 4
    zbufs = [nc.alloc_sbuf_tensor(f"zq{i}", [ZQ, D], F32R).ap() for i in range(4)]

    # PSUM: ps12(1) + psA(2) + psB(2) + y16 pool(2) + sp pool(1) = 8 banks
    ps12 = nc.alloc_psum_tensor("ps12", [72, D], F32).ap()
    psA = nc.alloc_psum_tensor("psA", [128, 2 * D], F32).ap()
    psB = nc.alloc_psum_tensor("psB", [97, 2 * D], F32).ap()

    with tile.TileContext(nc) as tc0:
        nc.sync.dma_start(out=cpk16, in_=cpack16)
        nc.sync.dma_start(out=cpk32, in_=cpack32)
        for zb in zbufs:
            nc.gpsimd.memset(zb[:, :].bitcast(F32), 0.0)

    with tile.TileContext(nc) as tc:
        with (
            tc.tile_pool(name="wt0p", bufs=3) as wt0p,
            tc.tile_pool(name="wt1p", bufs=3) as wt1p,
            tc.tile_pool(name="lnpool", bufs=2) as lnpool,
            tc.tile_pool(name="ps_y", bufs=2, space="PSUM") as ps_y,
            tc.tile_pool(name="ps_sp", bufs=1, space="PSUM") as ps_sp,
        ):
            def xdma(g):
                nc.sync.dma_start(
                    out=zbufs[g % 4][72:96, :],
                    in_=x_in[4 * g : 4 * g + 3].rearrange("n s d -> (n s) d"),
                )
                nc.sync.dma_start(
                    out=zbufs[g % 4][112:120, :],
                    in_=x_in[4 * g + 3 : 4 * g + 4].rearrange("n s d -> (n s) d"),
                )

            def s1(g):
                z = zbufs[g % 4]
                mm(ps12[0:16, :], ct("a1q"), z[0:120, :])
                nc.vector.tensor_scalar(
                    out=z[96:112, :], in0=ps12[0:16, :], scalar1=0.0, scalar2=None,
                    op0=AluOp.max,
                )

            def s2(g):
                z = zbufs[g % 4]
                mm(ps12[0:72, :], ct("a2q"), z[0:120, :])
                # NOTE: GPSIMD cannot read PSUM; >32-partition windows
                # must be naturally aligned (base 0 here)
                nc.scalar.activation(out=z[0:72, :], in_=ps12[0:72, :], func=Act.Relu)

            def emit_ln_stats(y16p):
                r = lnpool.tile([128, 2], F32, tag="r")
                sq = lnpool.tile([128, D], F32, tag="sq")
                nc.vector.tensor_reduce(
                    out=r[:, 0:1], in_=y16p[:], axis=mybir.AxisListType.X, op=AluOp.add
                )
                nc.scalar.activation(
                    out=sq[:], in_=y16p[:], func=Act.Square, accum_out=r[:, 1:2]
                )
                sp = ps_sp.tile([128, 2], F32, tag="sp")
                nc.tensor.matmul(sp[:], g16, r[:])
                mean = lnpool.tile([128, 1], F32, tag="mean")
                m2t = lnpool.tile([128, 1], F32, tag="m2t")
                var = lnpool.tile([128, 1], F32, tag="var")
                rstd = lnpool.tile([128, 1], F32, tag="rstd")
                nc.vector.tensor_scalar(
                    out=mean[:], in0=sp[:, 0:1], scalar1=1.0 / (H * D), scalar2=None,
                    op0=AluOp.mult,
                )
                nc.vector.tensor_tensor(out=m2t[:], in0=mean[:], in1=mean[:], op=AluOp.mult)
                nc.vector.scalar_tensor_tensor(
                    out=var[:], in0=sp[:, 1:2], scalar=1.0 / (H * D),
                    in1=m2t[:], op0=AluOp.mult, op1=AluOp.subtract,
                )
                nc.scalar.activation(out=rstd[:], in_=var[:], func=Act.Sqrt, bias=eps)
                nc.vector.reciprocal(out=rstd[:], in_=rstd[:])
                return mean, rstd

            def emit_ln_apply(blkp, y16p, mean, rstd):
                t1 = lnpool.tile([128, D], F32, tag="t1")
                nc.vector.tensor_scalar(
                    out=t1[:], in0=y16p[:], scalar1=mean[:], scalar2=rstd[:],
                    op0=AluOp.subtract, op1=AluOp.mult,
                )
                if apply_affine:
                    nc.gpsimd.tensor_tensor(out=t1[:], in0=t1[:], in1=lnw, op=AluOp.mult)
                    nc.gpsimd.tensor_tensor(out=t1[:], in0=t1[:], in1=lnb, op=AluOp.add)
                u = lnpool.tile([128, D], F32, tag="u")
                nc.vector.tensor_scalar(
                    out=u[:], in0=t1[:], scalar1=0.0, scalar2=pre[:, 0:1],
                    op0=AluOp.min, op1=AluOp.mult,
                )
                o16 = lnpool.tile([128, D], F32, tag="o16")
                nc.vector.scalar_tensor_tensor(
                    out=o16[:], in0=t1[:], scalar=0.0, in1=u[:],
                    op0=AluOp.max, op1=AluOp.add,
                )
                nc.sync.dma_start(
                    out=y_out[blkp * 16 : blkp * 16 + 16].rearrange("n h d -> (n h) d"),
                    in_=o16[:],
                )

            # software pipeline state
            prev = None  # (wt0, wt1, j0, y16, is_block_last)
            ln_stats_due = None  # (blk, y16)
            ln_apply_due = None  # (blk, y16, mean, rstd)

            # prologue: fill the DMA/S1/S2 pipeline for quads 0 and 1
            xdma(0)
            xdma(1)
            s1(0)
            s2(0)

            y16 = None
            for g in range(NQUADS):
                blk, Q = divmod(g, 4)
                z = zbufs[g % 4]
                if Q == 0:
                    y16 = ps_y.tile([128, D], F32, tag="y16")
                mm(y16[:], ct(f"c030q{Q}"), z[0:120, :], start=(Q == 0), stop=False)
                for p in range(2):
                    # lookahead: keep next quad's front work between this
                    # pair's matmuls so PE never waits on S1/S2 drains
                    if p == 0:
                        if g + 2 < NQUADS:
                            xdma(g + 2)
                        if g + 1 < NQUADS:
                            s1(g + 1)
                    else:
                        if g + 1 < NQUADS:
                            s2(g + 1)

                    # current pair S3 -> psA/psB, drains -> bf16 feature tiles
                    mm(psA[:, 0:D], ct(f"a3_{p}_a0"), z[0:120, :])
                    mm(psA[:, D : 2 * D], ct(f"a3_{p}_b0"), z[0:120, :])
                    mm(psB[:, 0:D], ct(f"a3_{p}_a1"), z[0:120, :])
                    mm(psB[:, D : 2 * D], ct(f"a3_{p}_b1"), z[0:120, :])
                    wt0 = wt0p.tile([128, 2 * D], F32R, tag="wt0")
                    wt1 = wt1p.tile([97, 2 * D], F32R, tag="wt1")
                    nc.vector.tensor_scalar(
                        out=wt0[:], in0=psA[:, :], scalar1=0.0, scalar2=None,
                        op0=AluOp.max,
                    )
                    nc.scalar.activation(out=wt1[:], in_=psB[:, :], func=Act.Relu)

                    # consume the previous pair's features
                    if prev is not None:
                        pw0, pw1, pj0, py16, plast = prev
                        mm(py16[:], ct(f"c8a{pj0}"), pw0[:, 0:D], start=False, stop=False)
                        mm(py16[:], ct(f"c8a{pj0 + 1}"), pw0[:, D : 2 * D], start=False, stop=False)
                        mm(py16[:], ct(f"c8b{pj0}"), pw1[:, 0:D], start=False, stop=False)
                        mm(py16[:], ct(f"c8b{pj0 + 1}"), pw1[:, D : 2 * D], start=False, stop=plast)
                        if plast:
                            ln_stats_due = (blk - 1, py16)
                    j0 = 4 * Q + 2 * p
                    prev = (wt0, wt1, j0, y16, j0 == 14)

                    # deferred LayerNorm of earlier blocks (vector-engine work)
                    if p == 1 and ln_apply_due is not None:
                        emit_ln_apply(*ln_apply_due)
                        ln_apply_due = None
                    if p == 1 and ln_stats_due is not None and ln_apply_due is None:
                        b_, y_ = ln_stats_due
                        mean, rstd = emit_ln_stats(y_)
                        ln_apply_due = (b_, y_, mean, rstd)
                        ln_stats_due = None

            # epilogue: flush last pair + remaining LayerNorms
            pw0, pw1, pj0, py16, plast = prev
            mm(py16[:], ct(f"c8a{pj0}"), pw0[:, 0:D], start=False, stop=False)
            mm(py16[:], ct(f"c8a{pj0 + 1}"), pw0[:, D : 2 * D], start=False, stop=False)
            mm(py16[:], ct(f"c8b{pj0}"), pw1[:, 0:D], start=False, stop=False)
            mm(py16[:], ct(f"c8b{pj0 + 1}"), pw1[:, D : 2 * D], start=False, stop=True)
            if ln_apply_due is not None:
                emit_ln_apply(*ln_apply_due)
            if ln_stats_due is not None:
                b_, y_ = ln_stats_due
                emit_ln_apply(b_, y_, *emit_ln_stats(y_))
            emit_ln_apply(NQUADS // 4 - 1, py16, *emit_ln_stats(py16))

    nc.compile()
    return nc


# --------------------------------------------------------------------------
# Entry point
# --------------------------------------------------------------------------
_CACHED = {}


def _get_module(apply_affine=False):
    key = ("nc", apply_affine)
    if key not in _CACHED:
        # cols16 layout is FM-independent; build with a dummy FM
        if "cols16" not in _CACHED:
            mats = _host_matrices(np.full((255, H), 1.0 / 8, np.float64))
            cp16, cols16 = _pack_consts16(mats)
            _CACHED["cols16"] = cols16
            _CACHED["cp16_width"] = cp16.shape[1]
        _CACHED[key] = build_module(
            apply_affine=apply_affine,
            cols16=_CACHED["cols16"],
            cp16_width=_CACHED["cp16_width"],
        )
    return _CACHED[key]


def kernel(x, FM, ln_weight, ln_bias, prelu_w):
    x16 = np.ascontiguousarray(np.asarray(x, np.float32))
    lnw = np.asarray(ln_weight, np.float32).reshape(H, D)
    lnb = np.asarray(ln_bias, np.float32).reshape(H, D)
    apply_affine = not (np.all(lnw == 1.0) and np.all(lnb == 0.0))
    lnw16 = np.tile(lnw.reshape(1, H, D), (16, 1, 1)).reshape(128, D)
    lnb16 = np.tile(lnb.reshape(1, H, D), (16, 1, 1)).reshape(128, D)

    mats = _host_matrices(np.asarray(FM, np.float64))
    cp16, cols16 = _pack_consts16(mats)
    _CACHED.setdefault("cols16", cols16)
    _CACHED.setdefault("cp16_width", cp16.shape[1])
    cp32 = _pack_consts32(lnw16, lnb16, float(np.asarray(prelu_w).reshape(-1)[0]))

    nc = _get_module(apply_affine)
    in_maps = []
    for c in range(NCORES):
        in_maps.append(
            {
                "x16": np.ascontiguousarray(x16[c * NPC : (c + 1) * NPC]),
                "cpack16": cp16,
                "cpack32": cp32,
            }
        )

    res = run_bass_kernel_spmd(nc, in_maps, core_ids=list(range(NCORES)))
    out = np.concatenate([r["y"] for r in res.results], axis=0)
    return out.astype(np.float32)


if __name__ == "__main__":
    # smoke-test build
    mats = _host_matrices(np.full((255, H), 1.0 / 8, np.float64))
    cp16, cols16 = _pack_consts16(mats)
    nc = build_module(npc=16, cols16=cols16, cp16_width=cp16.shape[1])
    print("module built ok")
